# revision 3
# baseline (speedup 1.0000x reference)
"""Causal multi-head attention (B=2, T=2048, E=1024, H=16) on 8 TRN2 NeuronCores.

Sharding: core c in 0..7 handles batch b = c//4 and head-group g = c%4
(4 heads, d-slice of 256 output channels). Each core:
  1. projects its Q/K/V slices (fp32r matmuls, contraction over E),
     producing QT/KT [d, t] layouts directly; V is PE-transposed into
     natural [t, d] layout with an appended ones column (for softmax sums).
  2. computes scores transposed ST[k, q] per head (contraction d=64; the
     two heads of each 128-row block are row-packed onto the PE array),
     exponentiates on the scalar engine (scale=1/8, no max subtraction:
     scores are O(1) here), and applies the causal mask only on diagonal
     tiles. Fully-masked k-tiles are skipped outright.
  3. accumulates OT[d+1, q] = [V|1]^T @ P^T over k-tiles in PSUM; row 64
     is the softmax denominator.
  4. normalizes via reciprocal + a PE broadcast matmul, then applies the
     output projection slice of Wo, writing a partial YT [E, T].
Host sums the 4 partials per batch and adds the bias.

All heavy matmuls run as float32r (TF32-like, ~11-bit mantissa, 1 cycle/row
at free-dim >= 256 vs 4 cycles/row for plain fp32).
"""

import sys

if "/opt/trn_rl_repo" not in sys.path:
    sys.path.insert(0, "/opt/trn_rl_repo")

import numpy as np

import concourse.bass as bass
import concourse.mybir as mybir
import concourse.tile as tile
from concourse import bacc
from concourse.masks import make_identity

F32 = mybir.dt.float32
F32R = mybir.dt.float32r
BF16 = mybir.dt.bfloat16
AF = mybir.ActivationFunctionType

B, T, E, H = 2, 2048, 1024, 16
D = E // H            # 64 head dim
P = 128               # partitions
DL = 256              # local d-slice per core (4 heads)
MT = DL // P          # 2 m-tiles
ET = E // P           # 8 e-tiles (contraction)
NKT = T // P          # 16 k-tiles
QC = 512              # q/t chunk width
NQC = T // QC         # 4 chunks

TRACE = False
LAST_RESULT = None

_BUILT = {}


def _build(causal: bool):
    nc = bacc.Bacc("TRN2", target_bir_lowering=False, debug=False)

    xq_d = nc.dram_tensor("xq", [E, T], F32R, kind="ExternalInput")
    xk_d = nc.dram_tensor("xk", [E, T], F32R, kind="ExternalInput")
    xv_d = nc.dram_tensor("xv", [E, T], F32R, kind="ExternalInput")
    wq_d = nc.dram_tensor("wq", [E, DL], F32R, kind="ExternalInput")
    wk_d = nc.dram_tensor("wk", [E, DL], F32R, kind="ExternalInput")
    wv_d = nc.dram_tensor("wv", [E, DL], F32R, kind="ExternalInput")
    wo_d = nc.dram_tensor("wo", [DL, E], F32R, kind="ExternalInput")
    sel_d = nc.dram_tensor("sel", [2 * MT * NQC, MT * NQC * P], F32,
                           kind="ExternalInput")
    ones_d = nc.dram_tensor("onescol", [P, NKT], F32R, kind="ExternalInput")
    if causal:
        dm_d = nc.dram_tensor("dmask", [4 * P, QC], F32R, kind="ExternalInput")
    yt_d = nc.dram_tensor("yt", [E, T], F32, kind="ExternalOutput")

    NU = MT * NQC  # 8 (m, qc) units

    with tile.TileContext(nc) as tc:
        with (
            tc.tile_pool(name="const", bufs=1) as cp,
            tc.tile_pool(name="xpool", bufs=3) as xp,
            tc.tile_pool(name="vtc", bufs=2) as vp,
            tc.tile_pool(name="pbuf", bufs=1) as pp,
            tc.tile_pool(name="stmp", bufs=2) as sp,
            tc.tile_pool(name="ysb", bufs=2) as yp,
            tc.tile_pool(name="psmm", bufs=3, space="PSUM") as pmm,
            tc.tile_pool(name="psav", bufs=4, space="PSUM") as pav,
            tc.tile_pool(name="pswm", bufs=1, space="PSUM") as pwm,
        ):
            # ---- constants / parameters in SBUF ----
            wq_sb = cp.tile([P, ET, DL], F32R, tag="wq")
            wk_sb = cp.tile([P, ET, DL], F32R, tag="wk")
            wv_sb = cp.tile([P, ET, DL], F32R, tag="wv")
            wo_sb = cp.tile([P, MT, E], F32R, tag="wo")
            sel_sb = cp.tile([2 * NU, NU * P], F32, tag="sel")
            ident = cp.tile([P, P], F32, tag="ident")
            nc.sync.dma_start(wq_sb[:], wq_d.ap().rearrange("(o p) f -> p o f", p=P))
            nc.sync.dma_start(wk_sb[:], wk_d.ap().rearrange("(o p) f -> p o f", p=P))
            nc.sync.dma_start(wv_sb[:], wv_d.ap().rearrange("(o p) f -> p o f", p=P))
            nc.sync.dma_start(wo_sb[:], wo_d.ap().rearrange("(o p) f -> p o f", p=P))
            nc.sync.dma_start(sel_sb[:], sel_d.ap())
            make_identity(nc, ident[:])

            # HAM clock-gate warmer: fp32r matmuls do not register as PE
            # activity, so the array stays throttled at 1.2 GHz. A tiny
            # bf16 matmul sprinkled into the stream keeps HAM at 8/8.
            warm_w = cp.tile([1, 1], BF16, tag="wrw")
            warm_x = cp.tile([1, 64], BF16, tag="wrx")
            nc.vector.memset(warm_w[:], 1.0)
            nc.vector.memset(warm_x[:], 1.0)
            warm_ps = pwm.tile([1, 64], F32, tag="wps")

            def warm():
                nc.tensor.matmul(warm_ps[:], warm_w[:], warm_x[:])
            if causal:
                dm_sb = cp.tile([P, 4, QC], F32R, tag="dm")
                nc.sync.dma_start(
                    dm_sb[:], dm_d.ap().rearrange("(o p) f -> p o f", p=P)
                )

            qt_sb = cp.tile([P, MT, T], F32R, tag="qt")
            kt_sb = cp.tile([P, MT, T], F32R, tag="kt")
            ot_sb = cp.tile([P, MT, T], F32R, tag="ot")
            vaug = cp.tile([P, MT, NKT, 132], F32R, tag="vaug")
            s_all = cp.tile([2 * NU, QC], F32, tag="sall")
            rcp_all = cp.tile([2 * NU, QC], F32, tag="rcp")

            # ones columns of vaug (col 64 for even head, col 130 for odd);
            # memset cannot emit float32r, so DMA from a ones input instead
            for m in range(MT):
                nc.sync.dma_start(vaug[:, m, :, 64], ones_d.ap())
                nc.sync.dma_start(vaug[:, m, :, 130], ones_d.ap())

            # ---- phase 1: projections ----
            def project(x_d, w_sb, dst, name, is_v=False):
                for tcc in range(NQC):
                    ps = [
                        pmm.tile([P, QC], F32, tag="mm", name=f"{name}_{tcc}_{m}")
                        for m in range(MT)
                    ]
                    for e in range(ET):
                        xch = xp.tile([P, QC], F32R, tag="xch")
                        nc.sync.dma_start(
                            xch[:],
                            x_d.ap()[e * P:(e + 1) * P, tcc * QC:(tcc + 1) * QC],
                        )
                        warm()
                        for m in range(MT):
                            nc.tensor.matmul(
                                ps[m][:],
                                w_sb[:, e, m * P:(m + 1) * P],
                                xch[:],
                                start=(e == 0),
                                stop=(e == ET - 1),
                            )
                    if not is_v:
                        for m in range(MT):
                            dsl = dst[:, m, tcc * QC:(tcc + 1) * QC]
                            if m == 0:
                                nc.vector.tensor_copy(dsl, ps[m][:])
                            else:
                                nc.scalar.copy(dsl, ps[m][:])
                    else:
                        # V: stage fp32, PE-transpose 128x128 blocks into vaug
                        for m in range(MT):
                            warm()
                            vtc = vp.tile([P, QC], F32, tag="vtc")
                            nc.vector.tensor_copy(vtc[:], ps[m][:])
                            for j in range(QC // P):
                                kt_i = tcc * (QC // P) + j
                                tr = pmm.tile([P, QC], F32, tag="mm")
                                nc.tensor.transpose(
                                    tr[:, 0:P], vtc[:, j * P:(j + 1) * P], ident[:]
                                )
                                nc.vector.tensor_copy(
                                    vaug[:, m, kt_i, 0:64], tr[:, 0:64]
                                )
                                nc.scalar.copy(
                                    vaug[:, m, kt_i, 66:130], tr[:, 64:128]
                                )

            project(xq_d, wq_sb, qt_sb, "q")
            project(xk_d, wk_sb, kt_sb, "k")
            project(xv_d, wv_sb, None, "v", is_v=True)

            # ---- phase 2: attention units ----
            for m in range(MT):
                for qc in range(NQC):
                    u = m * NQC + qc
                    K = 4 * (qc + 1) if causal else NKT
                    qsl = slice(qc * QC, (qc + 1) * QC)
                    pbe = pp.tile([P, NKT, QC], F32R, tag="pbe")
                    pbo = pp.tile([P, NKT, QC], F32R, tag="pbo")
                    for ki in range(K):
                        ksl = slice(ki * P, (ki + 1) * P)
                        warm()
                        pse = pmm.tile([P, QC], F32, tag="mm")
                        pso = pmm.tile([P, QC], F32, tag="mm")
                        nc.tensor.matmul(
                            pse[:], kt_sb[0:64, m, ksl], qt_sb[0:64, m, qsl]
                        )
                        nc.tensor.matmul(
                            pso[:], kt_sb[64:128, m, ksl], qt_sb[64:128, m, qsl]
                        )
                        nc.scalar.activation(
                            pbe[:, ki, :], pse[:], AF.Exp, scale=0.125
                        )
                        nc.scalar.activation(
                            pbo[:, ki, :], pso[:], AF.Exp, scale=0.125
                        )
                        if causal and ki >= 4 * qc:
                            pi = ki - 4 * qc
                            nc.vector.tensor_mul(
                                pbe[:, ki, :], pbe[:, ki, :], dm_sb[:, pi, :]
                            )
                            nc.vector.tensor_mul(
                                pbo[:, ki, :], pbo[:, ki, :], dm_sb[:, pi, :]
                            )
                    # AV accumulation: OT[d+1, q] per head parity
                    pe_av = pav.tile([P, QC], F32, tag="av")
                    po_av = pav.tile([P, QC], F32, tag="av")
                    for ki in range(K):
                        if ki % 2 == 0:
                            warm()
                        nc.tensor.matmul(
                            pe_av[0:65, :],
                            vaug[:, m, ki, 0:65],
                            pbe[:, ki, :],
                            start=(ki == 0),
                            stop=(ki == K - 1),
                        )
                    for ki in range(K):
                        if ki % 2 == 0:
                            warm()
                        nc.tensor.matmul(
                            po_av[0:65, :],
                            vaug[:, m, ki, 66:131],
                            pbo[:, ki, :],
                            start=(ki == 0),
                            stop=(ki == K - 1),
                        )
                    # stash raw outputs + softmax denominators
                    nc.vector.tensor_copy(ot_sb[0:64, m, qsl], pe_av[0:64, :])
                    nc.vector.tensor_copy(ot_sb[64:128, m, qsl], po_av[0:64, :])
                    ste = sp.tile([1, QC], F32, tag="st")
                    sto = sp.tile([1, QC], F32, tag="st")
                    nc.vector.tensor_copy(ste[:], pe_av[64:65, :])
                    nc.vector.tensor_copy(sto[:], po_av[64:65, :])
                    nc.sync.dma_start(s_all[2 * u:2 * u + 1, :], ste[:])
                    nc.sync.dma_start(s_all[2 * u + 1:2 * u + 2, :], sto[:])

            # ---- phase 3: normalize ----
            nc.vector.reciprocal(rcp_all[:], s_all[:])
            for m in range(MT):
                for qc in range(NQC):
                    u = m * NQC + qc
                    qsl = slice(qc * QC, (qc + 1) * QC)
                    pb = pmm.tile([P, QC], F32, tag="mm")
                    nc.tensor.matmul(
                        pb[:], sel_sb[:, u * P:(u + 1) * P], rcp_all[:]
                    )
                    nc.vector.tensor_mul(
                        ot_sb[:, m, qsl], ot_sb[:, m, qsl], pb[:]
                    )

            # ---- phase 4: output projection (partial) ----
            for mt in range(ET):
                for tcc in range(NQC):
                    py = pmm.tile([P, QC], F32, tag="mm")
                    warm()
                    for m2 in range(MT):
                        nc.tensor.matmul(
                            py[:],
                            wo_sb[:, m2, mt * P:(mt + 1) * P],
                            ot_sb[:, m2, tcc * QC:(tcc + 1) * QC],
                            start=(m2 == 0),
                            stop=(m2 == MT - 1),
                        )
                    yt_s = yp.tile([P, QC], F32, tag="y")
                    if tcc % 2 == 0:
                        nc.vector.tensor_copy(yt_s[:], py[:])
                    else:
                        nc.scalar.copy(yt_s[:], py[:])
                    nc.sync.dma_start(
                        yt_d.ap()[mt * P:(mt + 1) * P, tcc * QC:(tcc + 1) * QC],
                        yt_s[:],
                    )

    nc.compile()
    return nc


def _get_nc(causal: bool):
    if causal not in _BUILT:
        _BUILT[causal] = _build(causal)
    return _BUILT[causal]


def _sel_matrix():
    NU = MT * NQC
    sel = np.zeros((2 * NU, NU * P), np.float32)
    for u in range(NU):
        sel[2 * u, u * P:u * P + 64] = 1.0
        sel[2 * u + 1, u * P + 64:(u + 1) * P] = 1.0
    return sel


def _diag_masks():
    dm = np.zeros((4 * P, QC), np.float32)
    for p in range(4):
        r = np.arange(P)[:, None]
        q = np.arange(QC)[None, :]
        dm[p * P:(p + 1) * P] = (q >= p * P + r).astype(np.float32)
    return dm


def kernel(k, q, v, Wk, Wq, Wv, Wo, bo, mask):
    global LAST_RESULT
    from concourse.bass_utils import run_bass_kernel_spmd

    k = np.ascontiguousarray(np.asarray(k, np.float32))
    q = np.ascontiguousarray(np.asarray(q, np.float32))
    v = np.ascontiguousarray(np.asarray(v, np.float32))
    Wk = np.asarray(Wk, np.float32)
    Wq = np.asarray(Wq, np.float32)
    Wv = np.asarray(Wv, np.float32)
    Wo = np.asarray(Wo, np.float32)
    bo = np.asarray(bo, np.float32)
    mask = np.asarray(mask, bool)

    causal_ref = np.triu(np.ones((T, T), bool), k=1)
    if (mask == causal_ref).all():
        causal = True
    elif not mask.any():
        causal = False
    else:
        raise ValueError("only causal or all-false masks are supported")

    nc = _get_nc(causal)
    sel = _sel_matrix()
    dm = _diag_masks() if causal else None

    xq = [np.ascontiguousarray(q[b].T) for b in range(B)]
    xk = [np.ascontiguousarray(k[b].T) for b in range(B)]
    xv = [np.ascontiguousarray(v[b].T) for b in range(B)]

    in_maps = []
    for c in range(8):
        b, g = divmod(c, 4)
        ds = DL * g
        im = {
            "xq": xq[b],
            "xk": xk[b],
            "xv": xv[b],
            "wq": np.ascontiguousarray(Wq[ds:ds + DL, :].T),
            "wk": np.ascontiguousarray(Wk[ds:ds + DL, :].T),
            "wv": np.ascontiguousarray(Wv[ds:ds + DL, :].T),
            "wo": np.ascontiguousarray(Wo[:, ds:ds + DL].T),
            "sel": sel,
            "onescol": np.ones((P, NKT), np.float32),
        }
        if causal:
            im["dmask"] = dm
        in_maps.append(im)

    res = run_bass_kernel_spmd(nc, in_maps, list(range(8)), trace=TRACE)
    LAST_RESULT = res

    out = np.zeros((B, T, E), np.float32)
    for c in range(8):
        b = c // 4
        out[b] += res.results[c]["yt"].T
    out += bo
    return out


# revision 4
# speedup vs baseline: 1.0142x; 1.0142x over previous
"""Causal multi-head attention (B=2, T=2048, E=1024, H=16) on 8 TRN2 NeuronCores.

Sharding: core c in 0..7 handles batch b = c//4 and head-group g = c%4
(4 heads, d-slice of 256 output channels). Each core:
  1. projects its Q/K/V slices (fp32r matmuls, contraction over E),
     producing QT/KT [d, t] layouts directly; V is PE-transposed into
     natural [t, d] layout with an appended ones column (for softmax sums).
  2. computes scores transposed ST[k, q] per head (contraction d=64; the
     two heads of each 128-row block are row-packed onto the PE array),
     exponentiates on the scalar engine (scale=1/8, no max subtraction:
     scores are O(1) here), and applies the causal mask only on diagonal
     tiles. Fully-masked k-tiles are skipped outright.
  3. accumulates OT[d+1, q] = [V|1]^T @ P^T over k-tiles in PSUM; row 64
     is the softmax denominator.
  4. normalizes via reciprocal + a PE broadcast matmul, then applies the
     output projection slice of Wo, writing a partial YT [E, T].
Host sums the 4 partials per batch and adds the bias.

All heavy matmuls run as float32r (TF32-like, ~11-bit mantissa, 1 cycle/row
at free-dim >= 256 vs 4 cycles/row for plain fp32).
"""

import sys

if "/opt/trn_rl_repo" not in sys.path:
    sys.path.insert(0, "/opt/trn_rl_repo")

import numpy as np

import concourse.bass as bass
import concourse.mybir as mybir
import concourse.tile as tile
from concourse import bacc
from concourse.masks import make_identity

F32 = mybir.dt.float32
F32R = mybir.dt.float32r
BF16 = mybir.dt.bfloat16
AF = mybir.ActivationFunctionType

B, T, E, H = 2, 2048, 1024, 16
D = E // H            # 64 head dim
P = 128               # partitions
DL = 256              # local d-slice per core (4 heads)
MT = DL // P          # 2 m-tiles
ET = E // P           # 8 e-tiles (contraction)
NKT = T // P          # 16 k-tiles
QC = 512              # q/t chunk width
NQC = T // QC         # 4 chunks

TRACE = False
LAST_RESULT = None

_BUILT = {}


def _build(causal: bool):
    nc = bacc.Bacc("TRN2", target_bir_lowering=False, debug=False)

    xq_d = nc.dram_tensor("xq", [E, T], F32R, kind="ExternalInput")
    xk_d = nc.dram_tensor("xk", [E, T], F32R, kind="ExternalInput")
    xv_d = nc.dram_tensor("xv", [E, T], F32R, kind="ExternalInput")
    wq_d = nc.dram_tensor("wq", [E, DL], F32R, kind="ExternalInput")
    wk_d = nc.dram_tensor("wk", [E, DL], F32R, kind="ExternalInput")
    wv_d = nc.dram_tensor("wv", [E, DL], F32R, kind="ExternalInput")
    wo_d = nc.dram_tensor("wo", [DL, E], F32R, kind="ExternalInput")
    sel_d = nc.dram_tensor("sel", [2 * MT * NQC, MT * NQC * P], F32,
                           kind="ExternalInput")
    ones_d = nc.dram_tensor("onescol", [P, NKT], F32R, kind="ExternalInput")
    if causal:
        dm_d = nc.dram_tensor("dmask", [4 * P, QC], F32R, kind="ExternalInput")
    yt_d = nc.dram_tensor("yt", [E, T], F32, kind="ExternalOutput")

    NU = MT * NQC  # 8 (m, qc) units

    with tile.TileContext(nc) as tc:
        with (
            tc.tile_pool(name="const", bufs=1) as cp,
            tc.tile_pool(name="xpool", bufs=3) as xp,
            tc.tile_pool(name="vtc", bufs=2) as vp,
            tc.tile_pool(name="pbuf", bufs=1) as pp,
            tc.tile_pool(name="stmp", bufs=2) as sp,
            tc.tile_pool(name="ysb", bufs=2) as yp,
            tc.tile_pool(name="psmm", bufs=4, space="PSUM") as pmm,
            tc.tile_pool(name="psav", bufs=4, space="PSUM") as pav,
        ):
            # ---- constants / parameters in SBUF ----
            wq_sb = cp.tile([P, ET, DL], F32R, tag="wq")
            wk_sb = cp.tile([P, ET, DL], F32R, tag="wk")
            wv_sb = cp.tile([P, ET, DL], F32R, tag="wv")
            wo_sb = cp.tile([P, MT, E], F32R, tag="wo")
            sel_sb = cp.tile([2 * NU, NU * P], F32, tag="sel")
            ident = cp.tile([P, P], F32, tag="ident")
            nc.sync.dma_start(wq_sb[:], wq_d.ap().rearrange("(o p) f -> p o f", p=P))
            nc.sync.dma_start(wk_sb[:], wk_d.ap().rearrange("(o p) f -> p o f", p=P))
            nc.sync.dma_start(wv_sb[:], wv_d.ap().rearrange("(o p) f -> p o f", p=P))
            nc.sync.dma_start(wo_sb[:], wo_d.ap().rearrange("(o p) f -> p o f", p=P))
            nc.sync.dma_start(sel_sb[:], sel_d.ap())
            make_identity(nc, ident[:])
            if causal:
                dm_sb = cp.tile([P, 4, QC], F32R, tag="dm")
                nc.sync.dma_start(
                    dm_sb[:], dm_d.ap().rearrange("(o p) f -> p o f", p=P)
                )

            qt_sb = cp.tile([P, MT, T], F32R, tag="qt")
            kt_sb = cp.tile([P, MT, T], F32R, tag="kt")
            ot_sb = cp.tile([P, MT, T], F32R, tag="ot")
            vaug = cp.tile([P, MT, NKT, 132], F32R, tag="vaug")
            s_all = cp.tile([2 * NU, QC], F32, tag="sall")
            rcp_all = cp.tile([2 * NU, QC], F32, tag="rcp")

            # ones columns of vaug (col 64 for even head, col 130 for odd);
            # memset cannot emit float32r, so DMA from a ones input instead
            for m in range(MT):
                nc.sync.dma_start(vaug[:, m, :, 64], ones_d.ap())
                nc.sync.dma_start(vaug[:, m, :, 130], ones_d.ap())

            # ---- phase 1: projections ----
            def project(x_d, w_sb, dst, name, is_v=False):
                for tcc in range(NQC):
                    pool = pmm if tcc % 2 == 0 else pav
                    ptag = "mm" if tcc % 2 == 0 else "av"
                    ps = [
                        pool.tile([P, QC], F32, tag=ptag, name=f"{name}_{tcc}_{m}")
                        for m in range(MT)
                    ]
                    for e in range(ET):
                        xch = xp.tile([P, QC], F32R, tag="xch")
                        nc.sync.dma_start(
                            xch[:],
                            x_d.ap()[e * P:(e + 1) * P, tcc * QC:(tcc + 1) * QC],
                        )
                        for m in range(MT):
                            nc.tensor.matmul(
                                ps[m][:],
                                w_sb[:, e, m * P:(m + 1) * P],
                                xch[:],
                                start=(e == 0),
                                stop=(e == ET - 1),
                            )
                    if not is_v:
                        for m in range(MT):
                            dsl = dst[:, m, tcc * QC:(tcc + 1) * QC]
                            if m == 0:
                                nc.vector.tensor_copy(dsl, ps[m][:])
                            else:
                                nc.scalar.copy(dsl, ps[m][:])
                    else:
                        # V: stage fp32, PE-transpose 128x128 blocks into vaug
                        for m in range(MT):
                            vtc = vp.tile([P, QC], F32, tag="vtc")
                            nc.vector.tensor_copy(vtc[:], ps[m][:])
                            for j in range(QC // P):
                                kt_i = tcc * (QC // P) + j
                                tr = pmm.tile([P, QC], F32, tag="mm")
                                nc.tensor.transpose(
                                    tr[:, 0:P], vtc[:, j * P:(j + 1) * P], ident[:]
                                )
                                nc.vector.tensor_copy(
                                    vaug[:, m, kt_i, 0:64], tr[:, 0:64]
                                )
                                nc.scalar.copy(
                                    vaug[:, m, kt_i, 66:130], tr[:, 64:128]
                                )

            project(xq_d, wq_sb, qt_sb, "q")
            project(xk_d, wk_sb, kt_sb, "k")
            project(xv_d, wv_sb, None, "v", is_v=True)

            # ---- phase 2: attention units ----
            for m in range(MT):
                for qc in range(NQC):
                    u = m * NQC + qc
                    K = 4 * (qc + 1) if causal else NKT
                    qsl = slice(qc * QC, (qc + 1) * QC)
                    pbe = pp.tile([P, NKT, QC], F32R, tag="pbe")
                    pbo = pp.tile([P, NKT, QC], F32R, tag="pbo")
                    for ki in range(K):
                        ksl = slice(ki * P, (ki + 1) * P)
                        pse = pmm.tile([P, QC], F32, tag="mm")
                        pso = pmm.tile([P, QC], F32, tag="mm")
                        nc.tensor.matmul(
                            pse[:], kt_sb[0:64, m, ksl], qt_sb[0:64, m, qsl]
                        )
                        nc.tensor.matmul(
                            pso[:], kt_sb[64:128, m, ksl], qt_sb[64:128, m, qsl]
                        )
                        nc.scalar.activation(
                            pbe[:, ki, :], pse[:], AF.Exp, scale=0.125
                        )
                        nc.scalar.activation(
                            pbo[:, ki, :], pso[:], AF.Exp, scale=0.125
                        )
                        if causal and ki >= 4 * qc:
                            pi = ki - 4 * qc
                            nc.vector.tensor_mul(
                                pbe[:, ki, :], pbe[:, ki, :], dm_sb[:, pi, :]
                            )
                            nc.vector.tensor_mul(
                                pbo[:, ki, :], pbo[:, ki, :], dm_sb[:, pi, :]
                            )
                    # AV accumulation: OT[d+1, q] per head parity
                    pe_av = pav.tile([P, QC], F32, tag="av")
                    po_av = pav.tile([P, QC], F32, tag="av")
                    for ki in range(K):
                        nc.tensor.matmul(
                            pe_av[0:65, :],
                            vaug[:, m, ki, 0:65],
                            pbe[:, ki, :],
                            start=(ki == 0),
                            stop=(ki == K - 1),
                        )
                    for ki in range(K):
                        nc.tensor.matmul(
                            po_av[0:65, :],
                            vaug[:, m, ki, 66:131],
                            pbo[:, ki, :],
                            start=(ki == 0),
                            stop=(ki == K - 1),
                        )
                    # stash raw outputs + softmax denominators
                    nc.vector.tensor_copy(ot_sb[0:64, m, qsl], pe_av[0:64, :])
                    nc.vector.tensor_copy(ot_sb[64:128, m, qsl], po_av[0:64, :])
                    ste = sp.tile([1, QC], F32, tag="st")
                    sto = sp.tile([1, QC], F32, tag="st")
                    nc.vector.tensor_copy(ste[:], pe_av[64:65, :])
                    nc.vector.tensor_copy(sto[:], po_av[64:65, :])
                    nc.sync.dma_start(s_all[2 * u:2 * u + 1, :], ste[:])
                    nc.sync.dma_start(s_all[2 * u + 1:2 * u + 2, :], sto[:])

            # ---- phase 3+4: normalize per t-chunk, then its output proj ----
            nc.vector.reciprocal(rcp_all[:], s_all[:])
            for tcc in range(NQC):
                qsl = slice(tcc * QC, (tcc + 1) * QC)
                for m in range(MT):
                    u = m * NQC + tcc
                    pb = pmm.tile([P, QC], F32, tag="mm")
                    nc.tensor.matmul(
                        pb[:], sel_sb[:, u * P:(u + 1) * P], rcp_all[:]
                    )
                    nc.vector.tensor_mul(
                        ot_sb[:, m, qsl], ot_sb[:, m, qsl], pb[:]
                    )
                for mt in range(ET):
                    pool = pmm if mt % 2 == 0 else pav
                    ptag = "mm" if mt % 2 == 0 else "av"
                    py = pool.tile([P, QC], F32, tag=ptag)
                    for m2 in range(MT):
                        nc.tensor.matmul(
                            py[:],
                            wo_sb[:, m2, mt * P:(mt + 1) * P],
                            ot_sb[:, m2, qsl],
                            start=(m2 == 0),
                            stop=(m2 == MT - 1),
                        )
                    yt_s = yp.tile([P, QC], F32, tag="y")
                    if mt % 2 == 0:
                        nc.vector.tensor_copy(yt_s[:], py[:])
                    else:
                        nc.scalar.copy(yt_s[:], py[:])
                    nc.sync.dma_start(
                        yt_d.ap()[mt * P:(mt + 1) * P, qsl],
                        yt_s[:],
                    )

    nc.compile()
    return nc


def _get_nc(causal: bool):
    if causal not in _BUILT:
        _BUILT[causal] = _build(causal)
    return _BUILT[causal]


def _sel_matrix():
    NU = MT * NQC
    sel = np.zeros((2 * NU, NU * P), np.float32)
    for u in range(NU):
        sel[2 * u, u * P:u * P + 64] = 1.0
        sel[2 * u + 1, u * P + 64:(u + 1) * P] = 1.0
    return sel


def _diag_masks():
    dm = np.zeros((4 * P, QC), np.float32)
    for p in range(4):
        r = np.arange(P)[:, None]
        q = np.arange(QC)[None, :]
        dm[p * P:(p + 1) * P] = (q >= p * P + r).astype(np.float32)
    return dm


def kernel(k, q, v, Wk, Wq, Wv, Wo, bo, mask):
    global LAST_RESULT
    from concourse.bass_utils import run_bass_kernel_spmd

    k = np.ascontiguousarray(np.asarray(k, np.float32))
    q = np.ascontiguousarray(np.asarray(q, np.float32))
    v = np.ascontiguousarray(np.asarray(v, np.float32))
    Wk = np.asarray(Wk, np.float32)
    Wq = np.asarray(Wq, np.float32)
    Wv = np.asarray(Wv, np.float32)
    Wo = np.asarray(Wo, np.float32)
    bo = np.asarray(bo, np.float32)
    mask = np.asarray(mask, bool)

    causal_ref = np.triu(np.ones((T, T), bool), k=1)
    if (mask == causal_ref).all():
        causal = True
    elif not mask.any():
        causal = False
    else:
        raise ValueError("only causal or all-false masks are supported")

    nc = _get_nc(causal)
    sel = _sel_matrix()
    dm = _diag_masks() if causal else None

    xq = [np.ascontiguousarray(q[b].T) for b in range(B)]
    xk = [np.ascontiguousarray(k[b].T) for b in range(B)]
    xv = [np.ascontiguousarray(v[b].T) for b in range(B)]

    in_maps = []
    for c in range(8):
        b, g = divmod(c, 4)
        ds = DL * g
        im = {
            "xq": xq[b],
            "xk": xk[b],
            "xv": xv[b],
            "wq": np.ascontiguousarray(Wq[ds:ds + DL, :].T),
            "wk": np.ascontiguousarray(Wk[ds:ds + DL, :].T),
            "wv": np.ascontiguousarray(Wv[ds:ds + DL, :].T),
            "wo": np.ascontiguousarray(Wo[:, ds:ds + DL].T),
            "sel": sel,
            "onescol": np.ones((P, NKT), np.float32),
        }
        if causal:
            im["dmask"] = dm
        in_maps.append(im)

    res = run_bass_kernel_spmd(nc, in_maps, list(range(8)), trace=TRACE)
    LAST_RESULT = res

    out = np.zeros((B, T, E), np.float32)
    for c in range(8):
        b = c // 4
        out[b] += res.results[c]["yt"].T
    out += bo
    return out


# revision 5
# speedup vs baseline: 1.3140x; 1.2957x over previous
"""Causal multi-head attention (B=2, T=2048, E=1024, H=16) on 8 TRN2 NeuronCores.

Sharding: core c in 0..7 handles batch b = c//4 and head-group g = c%4
(4 heads, d-slice of 256 output channels). Each core:
  1. projects its Q/K/V slices (fp32r matmuls, contraction over E),
     producing QT/KT [d, t] layouts directly; V is PE-transposed into
     natural [t, d] layout with an appended ones column (for softmax sums).
  2. computes scores transposed ST[k, q] per head (contraction d=64; the
     two heads of each 128-row block are row-packed onto the PE array),
     exponentiates on the scalar engine (scale=1/8, no max subtraction:
     scores are O(1) here), and applies the causal mask only on diagonal
     tiles. Fully-masked k-tiles are skipped outright.
  3. accumulates OT[d+1, q] = [V|1]^T @ P^T over k-tiles in PSUM; row 64
     is the softmax denominator.
  4. normalizes via reciprocal + a PE broadcast matmul, then applies the
     output projection slice of Wo, writing a partial YT [E, T].
Host sums the 4 partials per batch and adds the bias.

All heavy matmuls run as float32r (TF32-like, ~11-bit mantissa, 1 cycle/row
at free-dim >= 256 vs 4 cycles/row for plain fp32).
"""

import sys

if "/opt/trn_rl_repo" not in sys.path:
    sys.path.insert(0, "/opt/trn_rl_repo")

import numpy as np

import concourse.bass as bass
import concourse.mybir as mybir
import concourse.tile as tile
from concourse import bacc
from concourse.masks import make_identity

F32 = mybir.dt.float32
F32R = mybir.dt.float32r
BF16 = mybir.dt.bfloat16
F16 = mybir.dt.float16
AF = mybir.ActivationFunctionType

B, T, E, H = 2, 2048, 1024, 16
D = E // H            # 64 head dim
P = 128               # partitions
DL = 256              # local d-slice per core (4 heads)
MT = DL // P          # 2 m-tiles
ET = E // P           # 8 e-tiles (contraction)
NKT = T // P          # 16 k-tiles
QC = 512              # q/t chunk width
NQC = T // QC         # 4 chunks

TRACE = False
LAST_RESULT = None

_BUILT = {}


def _build(causal: bool):
    nc = bacc.Bacc("TRN2", target_bir_lowering=False, debug=False)

    xq_d = nc.dram_tensor("xq", [E, T], F16, kind="ExternalInput")
    xk_d = nc.dram_tensor("xk", [E, T], F16, kind="ExternalInput")
    xv_d = nc.dram_tensor("xv", [E, T], F16, kind="ExternalInput")
    wq_d = nc.dram_tensor("wq", [E, DL], F16, kind="ExternalInput")
    wk_d = nc.dram_tensor("wk", [E, DL], F16, kind="ExternalInput")
    wv_d = nc.dram_tensor("wv", [E, DL], F16, kind="ExternalInput")
    wo_d = nc.dram_tensor("wo", [DL, E], F32R, kind="ExternalInput")
    sel_d = nc.dram_tensor("sel", [2 * MT * NQC, MT * NQC * P], F32,
                           kind="ExternalInput")
    ones_d = nc.dram_tensor("onescol", [P, NKT], F32R, kind="ExternalInput")
    if causal:
        dm_d = nc.dram_tensor("dmask", [4 * P, QC], F32R, kind="ExternalInput")
    yt_d = nc.dram_tensor("yt", [E, T], F32, kind="ExternalOutput")

    NU = MT * NQC  # 8 (m, qc) units

    with tile.TileContext(nc) as tc:
        with (
            tc.tile_pool(name="const", bufs=1) as cp,
            tc.tile_pool(name="xpool", bufs=6) as xp,
            tc.tile_pool(name="vtc", bufs=2) as vp,
            tc.tile_pool(name="pbuf", bufs=1) as pp,
            tc.tile_pool(name="stmp", bufs=2) as sp,
            tc.tile_pool(name="ysb", bufs=2) as yp,
            tc.tile_pool(name="psmm", bufs=4, space="PSUM") as pmm,
            tc.tile_pool(name="psav", bufs=4, space="PSUM") as pav,
        ):
            # ---- constants / parameters in SBUF ----
            wq_sb = cp.tile([P, ET, DL], F16, tag="wq")
            wk_sb = cp.tile([P, ET, DL], F16, tag="wk")
            wv_sb = cp.tile([P, ET, DL], F16, tag="wv")
            wo_sb = cp.tile([P, MT, E], F32R, tag="wo")
            sel_sb = cp.tile([2 * NU, NU * P], F32, tag="sel")
            ident = cp.tile([P, P], F32, tag="ident")
            nc.sync.dma_start(wq_sb[:], wq_d.ap().rearrange("(o p) f -> p o f", p=P))
            nc.sync.dma_start(wk_sb[:], wk_d.ap().rearrange("(o p) f -> p o f", p=P))
            nc.sync.dma_start(wv_sb[:], wv_d.ap().rearrange("(o p) f -> p o f", p=P))
            nc.sync.dma_start(wo_sb[:], wo_d.ap().rearrange("(o p) f -> p o f", p=P))
            nc.sync.dma_start(sel_sb[:], sel_d.ap())
            make_identity(nc, ident[:])
            if causal:
                dm_sb = cp.tile([P, 4, QC], F32R, tag="dm")
                nc.sync.dma_start(
                    dm_sb[:], dm_d.ap().rearrange("(o p) f -> p o f", p=P)
                )

            qt_sb = cp.tile([P, MT, T], F32R, tag="qt")
            kt_sb = cp.tile([P, MT, T], F32R, tag="kt")
            ot_sb = cp.tile([P, MT, T], F32R, tag="ot")
            vaug = cp.tile([P, MT, NKT, 132], F32R, tag="vaug")
            s_all = cp.tile([2 * NU, QC], F32, tag="sall")
            rcp_all = cp.tile([2 * NU, QC], F32, tag="rcp")

            # ones columns of vaug (col 64 for even head, col 130 for odd);
            # memset cannot emit float32r, so DMA from a ones input instead
            for m in range(MT):
                nc.sync.dma_start(vaug[:, m, :, 64], ones_d.ap())
                nc.sync.dma_start(vaug[:, m, :, 130], ones_d.ap())

            # ---- phase 1: projections ----
            def project(x_d, w_sb, dst, name, is_v=False):
                for tcc in range(NQC):
                    pool = pmm if tcc % 2 == 0 else pav
                    ptag = "mm" if tcc % 2 == 0 else "av"
                    ps = [
                        pool.tile([P, QC], F32, tag=ptag, name=f"{name}_{tcc}_{m}")
                        for m in range(MT)
                    ]
                    for e in range(ET):
                        xch = xp.tile([P, QC], F16, tag="xch")
                        dma_eng = nc.sync if e % 2 == 0 else nc.scalar
                        dma_eng.dma_start(
                            xch[:],
                            x_d.ap()[e * P:(e + 1) * P, tcc * QC:(tcc + 1) * QC],
                        )
                        for m in range(MT):
                            nc.tensor.matmul(
                                ps[m][:],
                                w_sb[:, e, m * P:(m + 1) * P],
                                xch[:],
                                start=(e == 0),
                                stop=(e == ET - 1),
                            )
                    if not is_v:
                        for m in range(MT):
                            dsl = dst[:, m, tcc * QC:(tcc + 1) * QC]
                            if m == 0:
                                nc.vector.tensor_copy(dsl, ps[m][:])
                            else:
                                nc.scalar.copy(dsl, ps[m][:])
                    else:
                        # V: stage fp32, PE-transpose 128x128 blocks into vaug
                        for m in range(MT):
                            vtc = vp.tile([P, QC], F32, tag="vtc")
                            nc.vector.tensor_copy(vtc[:], ps[m][:])
                            for j in range(QC // P):
                                kt_i = tcc * (QC // P) + j
                                tr = pmm.tile([P, QC], F32, tag="mm")
                                nc.tensor.transpose(
                                    tr[:, 0:P], vtc[:, j * P:(j + 1) * P], ident[:]
                                )
                                nc.vector.tensor_copy(
                                    vaug[:, m, kt_i, 0:64], tr[:, 0:64]
                                )
                                nc.scalar.copy(
                                    vaug[:, m, kt_i, 66:130], tr[:, 64:128]
                                )

            project(xq_d, wq_sb, qt_sb, "q")
            project(xk_d, wk_sb, kt_sb, "k")
            project(xv_d, wv_sb, None, "v", is_v=True)

            # ---- phase 2: attention units ----
            for m in range(MT):
                for qc in range(NQC):
                    u = m * NQC + qc
                    K = 4 * (qc + 1) if causal else NKT
                    qsl = slice(qc * QC, (qc + 1) * QC)
                    pbe = pp.tile([P, NKT, QC], F32R, tag="pbe")
                    pbo = pp.tile([P, NKT, QC], F32R, tag="pbo")
                    for ki in range(K):
                        ksl = slice(ki * P, (ki + 1) * P)
                        pse = pmm.tile([P, QC], F32, tag="mm")
                        pso = pmm.tile([P, QC], F32, tag="mm")
                        nc.tensor.matmul(
                            pse[:], kt_sb[0:64, m, ksl], qt_sb[0:64, m, qsl]
                        )
                        nc.tensor.matmul(
                            pso[:], kt_sb[64:128, m, ksl], qt_sb[64:128, m, qsl]
                        )
                        nc.scalar.activation(
                            pbe[:, ki, :], pse[:], AF.Exp, scale=0.125
                        )
                        nc.scalar.activation(
                            pbo[:, ki, :], pso[:], AF.Exp, scale=0.125
                        )
                        if causal and ki >= 4 * qc:
                            pi = ki - 4 * qc
                            nc.vector.tensor_mul(
                                pbe[:, ki, :], pbe[:, ki, :], dm_sb[:, pi, :]
                            )
                            nc.vector.tensor_mul(
                                pbo[:, ki, :], pbo[:, ki, :], dm_sb[:, pi, :]
                            )
                    # AV accumulation: OT[d+1, q] per head parity
                    pe_av = pav.tile([P, QC], F32, tag="av")
                    po_av = pav.tile([P, QC], F32, tag="av")
                    for ki in range(K):
                        nc.tensor.matmul(
                            pe_av[0:65, :],
                            vaug[:, m, ki, 0:65],
                            pbe[:, ki, :],
                            start=(ki == 0),
                            stop=(ki == K - 1),
                        )
                    for ki in range(K):
                        nc.tensor.matmul(
                            po_av[0:65, :],
                            vaug[:, m, ki, 66:131],
                            pbo[:, ki, :],
                            start=(ki == 0),
                            stop=(ki == K - 1),
                        )
                    # stash raw outputs + softmax denominators
                    nc.vector.tensor_copy(ot_sb[0:64, m, qsl], pe_av[0:64, :])
                    nc.vector.tensor_copy(ot_sb[64:128, m, qsl], po_av[0:64, :])
                    ste = sp.tile([1, QC], F32, tag="st")
                    sto = sp.tile([1, QC], F32, tag="st")
                    nc.vector.tensor_copy(ste[:], pe_av[64:65, :])
                    nc.vector.tensor_copy(sto[:], po_av[64:65, :])
                    nc.sync.dma_start(s_all[2 * u:2 * u + 1, :], ste[:])
                    nc.sync.dma_start(s_all[2 * u + 1:2 * u + 2, :], sto[:])

            # ---- phase 3+4: normalize per t-chunk, then its output proj ----
            nc.vector.reciprocal(rcp_all[:], s_all[:])
            for tcc in range(NQC):
                qsl = slice(tcc * QC, (tcc + 1) * QC)
                for m in range(MT):
                    u = m * NQC + tcc
                    pb = pmm.tile([P, QC], F32, tag="mm")
                    nc.tensor.matmul(
                        pb[:], sel_sb[:, u * P:(u + 1) * P], rcp_all[:]
                    )
                    nc.vector.tensor_mul(
                        ot_sb[:, m, qsl], ot_sb[:, m, qsl], pb[:]
                    )
                for mt in range(ET):
                    pool = pmm if mt % 2 == 0 else pav
                    ptag = "mm" if mt % 2 == 0 else "av"
                    py = pool.tile([P, QC], F32, tag=ptag)
                    for m2 in range(MT):
                        nc.tensor.matmul(
                            py[:],
                            wo_sb[:, m2, mt * P:(mt + 1) * P],
                            ot_sb[:, m2, qsl],
                            start=(m2 == 0),
                            stop=(m2 == MT - 1),
                        )
                    yt_s = yp.tile([P, QC], F32, tag="y")
                    if mt % 2 == 0:
                        nc.vector.tensor_copy(yt_s[:], py[:])
                    else:
                        nc.scalar.copy(yt_s[:], py[:])
                    (nc.sync if mt % 2 == 0 else nc.scalar).dma_start(
                        yt_d.ap()[mt * P:(mt + 1) * P, qsl],
                        yt_s[:],
                    )

    nc.compile()
    return nc


def _get_nc(causal: bool):
    if causal not in _BUILT:
        _BUILT[causal] = _build(causal)
    return _BUILT[causal]


def _sel_matrix():
    NU = MT * NQC
    sel = np.zeros((2 * NU, NU * P), np.float32)
    for u in range(NU):
        sel[2 * u, u * P:u * P + 64] = 1.0
        sel[2 * u + 1, u * P + 64:(u + 1) * P] = 1.0
    return sel


def _diag_masks():
    dm = np.zeros((4 * P, QC), np.float32)
    for p in range(4):
        r = np.arange(P)[:, None]
        q = np.arange(QC)[None, :]
        dm[p * P:(p + 1) * P] = (q >= p * P + r).astype(np.float32)
    return dm


def kernel(k, q, v, Wk, Wq, Wv, Wo, bo, mask):
    global LAST_RESULT
    from concourse.bass_utils import run_bass_kernel_spmd

    k = np.ascontiguousarray(np.asarray(k, np.float32))
    q = np.ascontiguousarray(np.asarray(q, np.float32))
    v = np.ascontiguousarray(np.asarray(v, np.float32))
    Wk = np.asarray(Wk, np.float32)
    Wq = np.asarray(Wq, np.float32)
    Wv = np.asarray(Wv, np.float32)
    Wo = np.asarray(Wo, np.float32)
    bo = np.asarray(bo, np.float32)
    mask = np.asarray(mask, bool)

    causal_ref = np.triu(np.ones((T, T), bool), k=1)
    if (mask == causal_ref).all():
        causal = True
    elif not mask.any():
        causal = False
    else:
        raise ValueError("only causal or all-false masks are supported")

    nc = _get_nc(causal)
    sel = _sel_matrix()
    dm = _diag_masks() if causal else None

    xq = [np.ascontiguousarray(q[b].T.astype(np.float16)) for b in range(B)]
    xk = [np.ascontiguousarray(k[b].T.astype(np.float16)) for b in range(B)]
    xv = [np.ascontiguousarray(v[b].T.astype(np.float16)) for b in range(B)]

    in_maps = []
    for c in range(8):
        b, g = divmod(c, 4)
        ds = DL * g
        im = {
            "xq": xq[b],
            "xk": xk[b],
            "xv": xv[b],
            "wq": np.ascontiguousarray(Wq[ds:ds + DL, :].T.astype(np.float16)),
            "wk": np.ascontiguousarray(Wk[ds:ds + DL, :].T.astype(np.float16)),
            "wv": np.ascontiguousarray(Wv[ds:ds + DL, :].T.astype(np.float16)),
            "wo": np.ascontiguousarray(Wo[:, ds:ds + DL].T),
            "sel": sel,
            "onescol": np.ones((P, NKT), np.float32),
        }
        if causal:
            im["dmask"] = dm
        in_maps.append(im)

    res = run_bass_kernel_spmd(nc, in_maps, list(range(8)), trace=TRACE)
    LAST_RESULT = res

    out = np.zeros((B, T, E), np.float32)
    for c in range(8):
        b = c // 4
        out[b] += res.results[c]["yt"].T
    out += bo
    return out


# revision 8
# speedup vs baseline: 1.7631x; 1.3418x over previous
"""Causal multi-head attention (B=2, T=2048, E=1024, H=16) on 8 TRN2 NeuronCores.

Sharding: core c handles batch c//4 and head-group c%4 (4 heads = a 256-wide
d-slice). Per core:
  1. Q/K/V projections (fp16 inputs, contraction over E, e-outer loops
     accumulating all 8 (m, t-chunk) PSUM groups per input; X streamed as
     full-T stripes so DMA rows are 4KB). QT/KT land in SBUF as fp16 [d, t];
     V is PE-transposed into [t, d] with an appended ones column.
  2. Per head: scores ST[k, q] (contraction d=64, two heads row-packed on
     the PE), exp on the scalar engine (scale=1/8; no max subtraction —
     scores here are O(1)), causal mask applied only on diagonal tiles,
     fully-masked tiles skipped. exp outputs go to per-k-tile fp32r tiles
     so the next unit's exps pipeline into this unit's AV reads.
  3. AV: OT[d+1, q] = [V|1]^T @ P^T accumulated over k-tiles in PSUM;
     row 64 is the softmax denominator.
  4. Normalize via reciprocal + PE broadcast (selection matmul), then the
     Wo slice produces a partial YT [E, T]; host sums 4 partials per batch
     and adds the bias.

Matmul dtypes: fp16 for projections/scores (inputs are fp16 anyway),
float32r (TF32-like, ~11-bit mantissa, full PE rate) for AV and the output
projection. PSUM accumulation is always fp32.
"""

import sys

if "/opt/trn_rl_repo" not in sys.path:
    sys.path.insert(0, "/opt/trn_rl_repo")

import numpy as np

import concourse.bass as bass
import concourse.mybir as mybir
import concourse.tile as tile
from concourse import bacc
from concourse.masks import make_identity

F32 = mybir.dt.float32
F32R = mybir.dt.float32r
F16 = mybir.dt.float16
AF = mybir.ActivationFunctionType

B, T, E, H = 2, 2048, 1024, 16
D = E // H            # 64 head dim
P = 128               # partitions
DL = 256              # local d-slice per core (4 heads)
MT = DL // P          # 2 m-tiles
ET = E // P           # 8 e-tiles (contraction)
NKT = T // P          # 16 k-tiles
QC = 512              # q/t chunk width
NQC = T // QC         # 4 chunks
PBUF = 20             # per-k-tile probability tiles per parity

TRACE = False
LAST_RESULT = None

_BUILT = {}


def _build(causal: bool):
    nc = bacc.Bacc("TRN2", target_bir_lowering=False, debug=False)

    xq_d = nc.dram_tensor("xq", [E, T], F16, kind="ExternalInput")
    xk_d = nc.dram_tensor("xk", [E, T], F16, kind="ExternalInput")
    xv_d = nc.dram_tensor("xv", [E, T], F16, kind="ExternalInput")
    wq_d = nc.dram_tensor("wq", [E, DL], F16, kind="ExternalInput")
    wk_d = nc.dram_tensor("wk", [E, DL], F16, kind="ExternalInput")
    wv_d = nc.dram_tensor("wv", [E, DL], F16, kind="ExternalInput")
    wo_d = nc.dram_tensor("wo", [DL, E], F32R, kind="ExternalInput")
    sel_d = nc.dram_tensor("sel", [2 * MT * NQC, MT * NQC * P], F32,
                           kind="ExternalInput")
    ones_d = nc.dram_tensor("onescol", [P, NKT], F32R, kind="ExternalInput")
    if causal:
        dm_d = nc.dram_tensor("dmask", [4 * P, QC], F32R, kind="ExternalInput")
    yt_d = nc.dram_tensor("yt", [E, T], F32, kind="ExternalOutput")

    NU = MT * NQC  # 8 (m, qc) units

    with tile.TileContext(nc) as tc:
        with (
            tc.tile_pool(name="const", bufs=1) as cp,
            tc.tile_pool(name="xpool", bufs=4) as xp,
            tc.tile_pool(name="vtc", bufs=4) as vp,
            tc.tile_pool(name="pbuf", bufs=PBUF) as pp,
            tc.tile_pool(name="stmp", bufs=2) as sp,
            tc.tile_pool(name="ysb", bufs=3) as yp,
            tc.tile_pool(name="psmm", bufs=4, space="PSUM") as pmm,
            tc.tile_pool(name="psav", bufs=4, space="PSUM") as pav,
        ):
            # ---- weights first (needed by the first projection) ----
            wq_sb = cp.tile([P, ET, DL], F16, tag="wq")
            wk_sb = cp.tile([P, ET, DL], F16, tag="wk")
            wv_sb = cp.tile([P, ET, DL], F16, tag="wv")
            nc.sync.dma_start(wq_sb[:], wq_d.ap().rearrange("(o p) f -> p o f", p=P))
            nc.scalar.dma_start(wk_sb[:], wk_d.ap().rearrange("(o p) f -> p o f", p=P))
            nc.scalar.dma_start(wv_sb[:], wv_d.ap().rearrange("(o p) f -> p o f", p=P))
            ident = cp.tile([P, P], F32, tag="ident")
            make_identity(nc, ident[:])

            qt_sb = cp.tile([P, MT, T], F16, tag="qt")
            kt_sb = cp.tile([P, MT, T], F16, tag="kt")
            ot_sb = cp.tile([P, MT, T], F32R, tag="ot")
            vaug = cp.tile([P, MT, NKT, 132], F32R, tag="vaug")
            s_all = cp.tile([2 * NU, QC], F32, tag="sall")
            rcp_all = cp.tile([2 * NU, QC], F32, tag="rcp")

            # ---- phase 1: projections (e-outer, full-T stripes) ----
            def project(x_d, w_sb, dst, is_v=False):
                ps = {}
                for m in range(MT):
                    for tcc in range(NQC):
                        u = m * NQC + tcc
                        pool, ptag = (pmm, "mm") if u < 4 else (pav, "av")
                        ps[m, tcc] = pool.tile([P, QC], F32, tag=ptag, name=f"ps_{m}_{tcc}")
                for e in range(ET):
                    xe = xp.tile([P, T], F16, tag="xe")
                    (nc.sync if e % 2 == 0 else nc.scalar).dma_start(
                        xe[:], x_d.ap()[e * P:(e + 1) * P, :]
                    )
                    for m in range(MT):
                        for tcc in range(NQC):
                            nc.tensor.matmul(
                                ps[m, tcc][:],
                                w_sb[:, e, m * P:(m + 1) * P],
                                xe[:, tcc * QC:(tcc + 1) * QC],
                                start=(e == 0),
                                stop=(e == ET - 1),
                            )
                if not is_v:
                    for m in range(MT):
                        for tcc in range(NQC):
                            dsl = dst[:, m, tcc * QC:(tcc + 1) * QC]
                            if m == 0:
                                nc.vector.tensor_copy(dsl, ps[m, tcc][:])
                            else:
                                nc.scalar.copy(dsl, ps[m, tcc][:])
                else:
                    # drain one PSUM pool at a time: copy its 4 accumulators
                    # to SBUF (freeing the banks), then run the transposes for
                    # that group out of the freed slots
                    for grp in range(2):
                        units = [
                            (m, tcc)
                            for m in range(MT)
                            for tcc in range(NQC)
                            if (m * NQC + tcc < 4) == (grp == 0)
                        ]
                        pool, ptag = (pmm, "mm") if grp == 0 else (pav, "av")
                        vtcs = {}
                        for m, tcc in units:
                            vtcs[m, tcc] = vp.tile(
                                [P, QC], F32, tag="vtc", name=f"vtc_{m}_{tcc}"
                            )
                            nc.vector.tensor_copy(vtcs[m, tcc][:], ps[m, tcc][:])
                        for m, tcc in units:
                            for j in range(QC // P):
                                kt_i = tcc * (QC // P) + j
                                tr = pool.tile([P, QC], F32, tag=ptag)
                                nc.tensor.transpose(
                                    tr[:, 0:P],
                                    vtcs[m, tcc][:, j * P:(j + 1) * P],
                                    ident[:],
                                )
                                nc.vector.tensor_copy(
                                    vaug[:, m, kt_i, 0:64], tr[:, 0:64]
                                )
                                nc.scalar.copy(
                                    vaug[:, m, kt_i, 66:130], tr[:, 64:128]
                                )

            project(xq_d, wq_sb, qt_sb)
            project(xk_d, wk_sb, kt_sb)
            # vaug ones columns + remaining parameters, loaded while q/k run
            for m in range(MT):
                nc.sync.dma_start(vaug[:, m, :, 64], ones_d.ap())
                nc.sync.dma_start(vaug[:, m, :, 130], ones_d.ap())
            wo_sb = cp.tile([P, MT, E], F32R, tag="wo")
            sel_sb = cp.tile([2 * NU, NU * P], F32, tag="sel")
            nc.sync.dma_start(wo_sb[:], wo_d.ap().rearrange("(o p) f -> p o f", p=P))
            nc.sync.dma_start(sel_sb[:], sel_d.ap())
            if causal:
                dm_sb = cp.tile([P, 4, QC], F32R, tag="dm")
                nc.sync.dma_start(
                    dm_sb[:], dm_d.ap().rearrange("(o p) f -> p o f", p=P)
                )
            project(xv_d, wv_sb, None, is_v=True)

            # ---- phase 2: attention units ----
            for m in range(MT):
                for qc in range(NQC):
                    u = m * NQC + qc
                    K = 4 * (qc + 1) if causal else NKT
                    qsl = slice(qc * QC, (qc + 1) * QC)
                    pbes, pbos = [], []
                    for ki in range(K):
                        ksl = slice(ki * P, (ki + 1) * P)
                        pse = pmm.tile([P, QC], F32, tag="mm")
                        pso = pmm.tile([P, QC], F32, tag="mm")
                        nc.tensor.matmul(
                            pse[:], kt_sb[0:64, m, ksl], qt_sb[0:64, m, qsl]
                        )
                        nc.tensor.matmul(
                            pso[:], kt_sb[64:128, m, ksl], qt_sb[64:128, m, qsl]
                        )
                        pbe = pp.tile([P, QC], F32R, tag="pbe")
                        pbo = pp.tile([P, QC], F32R, tag="pbo")
                        nc.scalar.activation(pbe[:], pse[:], AF.Exp, scale=0.125)
                        nc.scalar.activation(pbo[:], pso[:], AF.Exp, scale=0.125)
                        if causal and ki >= 4 * qc:
                            pi = ki - 4 * qc
                            nc.vector.tensor_mul(pbe[:], pbe[:], dm_sb[:, pi, :])
                            nc.vector.tensor_mul(pbo[:], pbo[:], dm_sb[:, pi, :])
                        pbes.append(pbe)
                        pbos.append(pbo)
                    pe_av = pav.tile([P, QC], F32, tag="av")
                    po_av = pav.tile([P, QC], F32, tag="av")
                    for ki in range(K):
                        nc.tensor.matmul(
                            pe_av[0:65, :],
                            vaug[:, m, ki, 0:65],
                            pbes[ki][:],
                            start=(ki == 0),
                            stop=(ki == K - 1),
                        )
                    for ki in range(K):
                        nc.tensor.matmul(
                            po_av[0:65, :],
                            vaug[:, m, ki, 66:131],
                            pbos[ki][:],
                            start=(ki == 0),
                            stop=(ki == K - 1),
                        )
                    nc.vector.tensor_copy(ot_sb[0:64, m, qsl], pe_av[0:64, :])
                    nc.vector.tensor_copy(ot_sb[64:128, m, qsl], po_av[0:64, :])
                    ste = sp.tile([1, QC], F32, tag="st")
                    sto = sp.tile([1, QC], F32, tag="st")
                    nc.vector.tensor_copy(ste[:], pe_av[64:65, :])
                    nc.vector.tensor_copy(sto[:], po_av[64:65, :])
                    nc.sync.dma_start(s_all[2 * u:2 * u + 1, :], ste[:])
                    nc.sync.dma_start(s_all[2 * u + 1:2 * u + 2, :], sto[:])

            # ---- phase 3+4: normalize per t-chunk, then its output proj ----
            nc.vector.reciprocal(rcp_all[:], s_all[:])
            for tcc in range(NQC):
                qsl = slice(tcc * QC, (tcc + 1) * QC)
                for m in range(MT):
                    u = m * NQC + tcc
                    pb = pmm.tile([P, QC], F32, tag="mm")
                    nc.tensor.matmul(
                        pb[:], sel_sb[:, u * P:(u + 1) * P], rcp_all[:]
                    )
                    nc.vector.tensor_mul(
                        ot_sb[:, m, qsl], ot_sb[:, m, qsl], pb[:]
                    )
                for mt in range(ET):
                    pool, ptag = (pmm, "mm") if mt % 2 == 0 else (pav, "av")
                    py = pool.tile([P, QC], F32, tag=ptag)
                    for m2 in range(MT):
                        nc.tensor.matmul(
                            py[:],
                            wo_sb[:, m2, mt * P:(mt + 1) * P],
                            ot_sb[:, m2, qsl],
                            start=(m2 == 0),
                            stop=(m2 == MT - 1),
                        )
                    yt_s = yp.tile([P, QC], F32, tag="y")
                    if mt % 2 == 0:
                        nc.vector.tensor_copy(yt_s[:], py[:])
                    else:
                        nc.scalar.copy(yt_s[:], py[:])
                    (nc.sync if mt % 2 == 0 else nc.scalar).dma_start(
                        yt_d.ap()[mt * P:(mt + 1) * P, qsl],
                        yt_s[:],
                    )

    nc.compile()
    return nc


def _get_nc(causal: bool):
    if causal not in _BUILT:
        _BUILT[causal] = _build(causal)
    return _BUILT[causal]


def _sel_matrix():
    NU = MT * NQC
    sel = np.zeros((2 * NU, NU * P), np.float32)
    for u in range(NU):
        sel[2 * u, u * P:u * P + 64] = 1.0
        sel[2 * u + 1, u * P + 64:(u + 1) * P] = 1.0
    return sel


def _diag_masks():
    dm = np.zeros((4 * P, QC), np.float32)
    for p in range(4):
        r = np.arange(P)[:, None]
        q = np.arange(QC)[None, :]
        dm[p * P:(p + 1) * P] = (q >= p * P + r).astype(np.float32)
    return dm


def kernel(k, q, v, Wk, Wq, Wv, Wo, bo, mask):
    global LAST_RESULT
    from concourse.bass_utils import run_bass_kernel_spmd

    k = np.ascontiguousarray(np.asarray(k, np.float32))
    q = np.ascontiguousarray(np.asarray(q, np.float32))
    v = np.ascontiguousarray(np.asarray(v, np.float32))
    Wk = np.asarray(Wk, np.float32)
    Wq = np.asarray(Wq, np.float32)
    Wv = np.asarray(Wv, np.float32)
    Wo = np.asarray(Wo, np.float32)
    bo = np.asarray(bo, np.float32)
    mask = np.asarray(mask, bool)

    causal_ref = np.triu(np.ones((T, T), bool), k=1)
    if (mask == causal_ref).all():
        causal = True
    elif not mask.any():
        causal = False
    else:
        raise ValueError("only causal or all-false masks are supported")

    nc = _get_nc(causal)
    sel = _sel_matrix()
    dm = _diag_masks() if causal else None

    xq = [np.ascontiguousarray(q[b].T.astype(np.float16)) for b in range(B)]
    xk = [np.ascontiguousarray(k[b].T.astype(np.float16)) for b in range(B)]
    xv = [np.ascontiguousarray(v[b].T.astype(np.float16)) for b in range(B)]

    in_maps = []
    for c in range(8):
        b, g = divmod(c, 4)
        ds = DL * g
        im = {
            "xq": xq[b],
            "xk": xk[b],
            "xv": xv[b],
            "wq": np.ascontiguousarray(Wq[ds:ds + DL, :].T.astype(np.float16)),
            "wk": np.ascontiguousarray(Wk[ds:ds + DL, :].T.astype(np.float16)),
            "wv": np.ascontiguousarray(Wv[ds:ds + DL, :].T.astype(np.float16)),
            "wo": np.ascontiguousarray(Wo[:, ds:ds + DL].T),
            "sel": sel,
            "onescol": np.ones((P, NKT), np.float32),
        }
        if causal:
            im["dmask"] = dm
        in_maps.append(im)

    res = run_bass_kernel_spmd(nc, in_maps, list(range(8)), trace=TRACE)
    LAST_RESULT = res

    out = np.zeros((B, T, E), np.float32)
    for c in range(8):
        b = c // 4
        out[b] += res.results[c]["yt"].T
    out += bo
    return out


# revision 9
# speedup vs baseline: 2.1797x; 1.2363x over previous
"""Causal multi-head attention (B=2, T=2048, E=1024, H=16) on 8 TRN2 NeuronCores.

Sharding: core c handles batch c//4 and head-group c%4 (4 heads = a 256-wide
d-slice). Per core:
  1. Q/K/V projections (fp16 inputs, contraction over E, e-outer loops
     accumulating all 8 (m, t-chunk) PSUM groups per input; X streamed as
     full-T stripes so DMA rows are 4KB). QT/KT land in SBUF as fp16 [d, t];
     V is PE-transposed into [t, d] with an appended ones column.
  2. Per head: scores ST[k, q] (contraction d=64, two heads row-packed on
     the PE), exp on the scalar engine (scale=1/8; no max subtraction —
     scores here are O(1)), causal mask applied only on diagonal tiles,
     fully-masked tiles skipped. exp outputs go to per-k-tile fp32r tiles
     so the next unit's exps pipeline into this unit's AV reads.
  3. AV: OT[d+1, q] = [V|1]^T @ P^T accumulated over k-tiles in PSUM;
     row 64 is the softmax denominator.
  4. Normalize via reciprocal + PE broadcast (selection matmul), then the
     Wo slice produces a partial YT [E, T]; host sums 4 partials per batch
     and adds the bias.

Matmul dtypes: fp16 for projections/scores (inputs are fp16 anyway),
float32r (TF32-like, ~11-bit mantissa, full PE rate) for AV and the output
projection. PSUM accumulation is always fp32.
"""

import sys

if "/opt/trn_rl_repo" not in sys.path:
    sys.path.insert(0, "/opt/trn_rl_repo")

import numpy as np

import concourse.bass as bass
import concourse.mybir as mybir
import concourse.tile as tile
from concourse import bacc
from concourse.masks import make_identity

F32 = mybir.dt.float32
F32R = mybir.dt.float32r
F16 = mybir.dt.float16
AF = mybir.ActivationFunctionType

B, T, E, H = 2, 2048, 1024, 16
D = E // H            # 64 head dim
P = 128               # partitions
DL = 256              # local d-slice per core (4 heads)
MT = DL // P          # 2 m-tiles
ET = E // P           # 8 e-tiles (contraction)
NKT = T // P          # 16 k-tiles
QC = 512              # q/t chunk width
NQC = T // QC         # 4 chunks
PBUF = 20             # per-k-tile probability pair tiles

TRACE = False
LAST_RESULT = None

_BUILT = {}


def _build(causal: bool):
    nc = bacc.Bacc("TRN2", target_bir_lowering=False, debug=False)

    xq_d = nc.dram_tensor("xq", [E, T], F16, kind="ExternalInput")
    xk_d = nc.dram_tensor("xk", [E, T], F16, kind="ExternalInput")
    xv_d = nc.dram_tensor("xv", [E, T], F16, kind="ExternalInput")
    wq_d = nc.dram_tensor("wq", [E, DL], F16, kind="ExternalInput")
    wk_d = nc.dram_tensor("wk", [E, DL], F16, kind="ExternalInput")
    wv_d = nc.dram_tensor("wv", [E, DL], F16, kind="ExternalInput")
    wo_d = nc.dram_tensor("wo", [DL, E], F32R, kind="ExternalInput")
    sel_d = nc.dram_tensor("sel", [2 * MT * NQC, MT * NQC * P], F32,
                           kind="ExternalInput")
    ones_d = nc.dram_tensor("onescol", [P, NKT], F32R, kind="ExternalInput")
    if causal:
        dm_d = nc.dram_tensor("dmask", [4 * P, QC], F32R, kind="ExternalInput")
    yt_d = nc.dram_tensor("yt", [E, T], F32, kind="ExternalOutput")

    NU = MT * NQC  # 8 (m, qc) units

    with tile.TileContext(nc) as tc:
        with (
            tc.tile_pool(name="const", bufs=1) as cp,
            tc.tile_pool(name="xpool", bufs=4) as xp,
            tc.tile_pool(name="vtc", bufs=4) as vp,
            tc.tile_pool(name="pbuf", bufs=PBUF) as pp,
            tc.tile_pool(name="stmp", bufs=2) as sp,
            tc.tile_pool(name="ysb", bufs=3) as yp,
            tc.tile_pool(name="psmm", bufs=2, space="PSUM") as pmm,
            tc.tile_pool(name="psav", bufs=4, space="PSUM") as pav,
        ):
            # ---- weights first (needed by the first projection) ----
            wq_sb = cp.tile([P, ET, DL], F16, tag="wq")
            wk_sb = cp.tile([P, ET, DL], F16, tag="wk")
            wv_sb = cp.tile([P, ET, DL], F16, tag="wv")
            nc.sync.dma_start(wq_sb[:], wq_d.ap().rearrange("(o p) f -> p o f", p=P))
            nc.scalar.dma_start(wk_sb[:], wk_d.ap().rearrange("(o p) f -> p o f", p=P))
            nc.scalar.dma_start(wv_sb[:], wv_d.ap().rearrange("(o p) f -> p o f", p=P))
            ident = cp.tile([P, P], F32, tag="ident")
            make_identity(nc, ident[:])

            qt_sb = cp.tile([P, MT, T], F16, tag="qt")
            kt_sb = cp.tile([P, MT, T], F16, tag="kt")
            ot_sb = cp.tile([P, MT, T], F32R, tag="ot")
            vaug = cp.tile([P, MT, NKT, 132], F32R, tag="vaug")
            s_all = cp.tile([2 * NU, QC], F32, tag="sall")
            rcp_all = cp.tile([2 * NU, QC], F32, tag="rcp")

            # ---- phase 1: projections (e-outer, full-T stripes) ----
            def project(x_d, w_sb, dst, is_v=False):
                ps = {}
                pairs = [pmm.tile([P, 2, QC], F32, tag="mm", name=f"pp_{i}")
                         for i in range(2)]
                for m in range(MT):
                    for tcc in range(NQC):
                        u = m * NQC + tcc
                        if u < 4:
                            ps[m, tcc] = pairs[u // 2][:, u % 2, :]
                        else:
                            ps[m, tcc] = pav.tile(
                                [P, QC], F32, tag="av", name=f"ps_{m}_{tcc}"
                            )
                for e in range(ET):
                    xe = xp.tile([P, T], F16, tag="xe")
                    (nc.sync if e % 2 == 0 else nc.scalar).dma_start(
                        xe[:], x_d.ap()[e * P:(e + 1) * P, :]
                    )
                    for m in range(MT):
                        for tcc in range(NQC):
                            nc.tensor.matmul(
                                ps[m, tcc][:] if hasattr(ps[m, tcc], "tile") else ps[m, tcc],
                                w_sb[:, e, m * P:(m + 1) * P],
                                xe[:, tcc * QC:(tcc + 1) * QC],
                                start=(e == 0),
                                stop=(e == ET - 1),
                            )
                if not is_v:
                    for m in range(MT):
                        for tcc in range(NQC):
                            dsl = dst[:, m, tcc * QC:(tcc + 1) * QC]
                            if m == 0:
                                nc.vector.tensor_copy(dsl, ps[m, tcc][:] if hasattr(ps[m, tcc], "tile") else ps[m, tcc])
                            else:
                                nc.scalar.copy(dsl, ps[m, tcc][:] if hasattr(ps[m, tcc], "tile") else ps[m, tcc])
                else:
                    # drain one PSUM pool at a time: copy its 4 accumulators
                    # to SBUF (freeing the banks), then run the transposes for
                    # that group out of the freed slots
                    for grp in range(2):
                        units = [
                            (m, tcc)
                            for m in range(MT)
                            for tcc in range(NQC)
                            if (m * NQC + tcc < 4) == (grp == 0)
                        ]
                        pool, ptag = (pmm, "mm") if grp == 0 else (pav, "av")
                        trshape = [P, 2, QC] if grp == 0 else [P, QC]
                        vtcs = {}
                        for m, tcc in units:
                            vtcs[m, tcc] = vp.tile(
                                [P, QC], F32, tag="vtc", name=f"vtc_{m}_{tcc}"
                            )
                            nc.vector.tensor_copy(vtcs[m, tcc][:], ps[m, tcc][:] if hasattr(ps[m, tcc], "tile") else ps[m, tcc])
                        for m, tcc in units:
                            for j in range(QC // P):
                                kt_i = tcc * (QC // P) + j
                                tr_t = pool.tile(trshape, F32, tag=ptag, name="tr")
                                tr = tr_t[:, 0, :] if grp == 0 else tr_t[:]
                                nc.tensor.transpose(
                                    tr[:, 0:P],
                                    vtcs[m, tcc][:, j * P:(j + 1) * P],
                                    ident[:],
                                )
                                nc.vector.tensor_copy(
                                    vaug[:, m, kt_i, 0:64], tr[:, 0:64]
                                )
                                nc.scalar.copy(
                                    vaug[:, m, kt_i, 66:130], tr[:, 64:128]
                                )

            project(xq_d, wq_sb, qt_sb)
            project(xk_d, wk_sb, kt_sb)
            # vaug ones columns + remaining parameters, loaded while q/k run
            for m in range(MT):
                nc.sync.dma_start(vaug[:, m, :, 64], ones_d.ap())
                nc.sync.dma_start(vaug[:, m, :, 130], ones_d.ap())
            wo_sb = cp.tile([P, MT, E], F32R, tag="wo")
            sel_sb = cp.tile([2 * NU, NU * P], F32, tag="sel")
            nc.sync.dma_start(wo_sb[:], wo_d.ap().rearrange("(o p) f -> p o f", p=P))
            nc.sync.dma_start(sel_sb[:], sel_d.ap())
            if causal:
                dm_sb = cp.tile([P, 4, QC], F32R, tag="dm")
                nc.sync.dma_start(
                    dm_sb[:], dm_d.ap().rearrange("(o p) f -> p o f", p=P)
                )
            project(xv_d, wv_sb, None, is_v=True)

            # ---- phase 2: attention units ----
            for m in range(MT):
                for qc in range(NQC):
                    u = m * NQC + qc
                    K = 4 * (qc + 1) if causal else NKT
                    qsl = slice(qc * QC, (qc + 1) * QC)
                    pbs = []
                    for ki in range(K):
                        ksl = slice(ki * P, (ki + 1) * P)
                        psc = pmm.tile([P, 2, QC], F32, tag="mm", name="psc")
                        nc.tensor.matmul(
                            psc[:, 0, :], kt_sb[0:64, m, ksl], qt_sb[0:64, m, qsl]
                        )
                        nc.tensor.matmul(
                            psc[:, 1, :], kt_sb[64:128, m, ksl], qt_sb[64:128, m, qsl]
                        )
                        pb2 = pp.tile([P, 2, QC], F32R, tag="pb2")
                        nc.scalar.activation(pb2[:], psc[:], AF.Exp, scale=0.125)
                        if causal and ki >= 4 * qc:
                            # mask is all-ones beyond col 128*(pi+1); multiply
                            # only the prefix that can contain zeros
                            pi = ki - 4 * qc
                            w = P * (pi + 1)
                            for par in range(2):
                                nc.vector.tensor_mul(
                                    pb2[:, par, 0:w],
                                    pb2[:, par, 0:w],
                                    dm_sb[:, pi, 0:w],
                                )
                        pbs.append(pb2)
                    pe_av = pav.tile([P, QC], F32, tag="av")
                    po_av = pav.tile([P, QC], F32, tag="av")
                    for ki in range(K):
                        nc.tensor.matmul(
                            pe_av[0:65, :],
                            vaug[:, m, ki, 0:65],
                            pbs[ki][:, 0, :],
                            start=(ki == 0),
                            stop=(ki == K - 1),
                        )
                    for ki in range(K):
                        nc.tensor.matmul(
                            po_av[0:65, :],
                            vaug[:, m, ki, 66:131],
                            pbs[ki][:, 1, :],
                            start=(ki == 0),
                            stop=(ki == K - 1),
                        )
                    nc.vector.tensor_copy(ot_sb[0:64, m, qsl], pe_av[0:64, :])
                    nc.vector.tensor_copy(ot_sb[64:128, m, qsl], po_av[0:64, :])
                    ste = sp.tile([1, QC], F32, tag="st")
                    sto = sp.tile([1, QC], F32, tag="st")
                    nc.vector.tensor_copy(ste[:], pe_av[64:65, :])
                    nc.vector.tensor_copy(sto[:], po_av[64:65, :])
                    nc.sync.dma_start(s_all[2 * u:2 * u + 1, :], ste[:])
                    nc.sync.dma_start(s_all[2 * u + 1:2 * u + 2, :], sto[:])

            # ---- phase 3+4: normalize per t-chunk, then its output proj ----
            nc.vector.reciprocal(rcp_all[:], s_all[:])
            for tcc in range(NQC):
                qsl = slice(tcc * QC, (tcc + 1) * QC)
                for m in range(MT):
                    u = m * NQC + tcc
                    pb_t = pmm.tile([P, 2, QC], F32, tag="mm", name="pbt")
                    pb = pb_t[:, 0, :]
                    nc.tensor.matmul(
                        pb, sel_sb[:, u * P:(u + 1) * P], rcp_all[:]
                    )
                    nc.vector.tensor_mul(
                        ot_sb[:, m, qsl], ot_sb[:, m, qsl], pb
                    )
                for mt in range(ET):
                    if mt % 2 == 0:
                        py = pmm.tile([P, 2, QC], F32, tag="mm", name="pyt")[:, 0, :]
                    else:
                        py = pav.tile([P, QC], F32, tag="av")
                    for m2 in range(MT):
                        nc.tensor.matmul(
                            py,
                            wo_sb[:, m2, mt * P:(mt + 1) * P],
                            ot_sb[:, m2, qsl],
                            start=(m2 == 0),
                            stop=(m2 == MT - 1),
                        )
                    yt_s = yp.tile([P, QC], F32, tag="y")
                    if mt % 2 == 0:
                        nc.vector.tensor_copy(yt_s[:], py)
                    else:
                        nc.scalar.copy(yt_s[:], py)
                    (nc.sync if mt % 2 == 0 else nc.scalar).dma_start(
                        yt_d.ap()[mt * P:(mt + 1) * P, qsl],
                        yt_s[:],
                    )

    nc.compile()
    return nc


def _get_nc(causal: bool):
    if causal not in _BUILT:
        _BUILT[causal] = _build(causal)
    return _BUILT[causal]


def _sel_matrix():
    NU = MT * NQC
    sel = np.zeros((2 * NU, NU * P), np.float32)
    for u in range(NU):
        sel[2 * u, u * P:u * P + 64] = 1.0
        sel[2 * u + 1, u * P + 64:(u + 1) * P] = 1.0
    return sel


def _diag_masks():
    dm = np.zeros((4 * P, QC), np.float32)
    for p in range(4):
        r = np.arange(P)[:, None]
        q = np.arange(QC)[None, :]
        dm[p * P:(p + 1) * P] = (q >= p * P + r).astype(np.float32)
    return dm


def kernel(k, q, v, Wk, Wq, Wv, Wo, bo, mask):
    global LAST_RESULT
    from concourse.bass_utils import run_bass_kernel_spmd

    k = np.ascontiguousarray(np.asarray(k, np.float32))
    q = np.ascontiguousarray(np.asarray(q, np.float32))
    v = np.ascontiguousarray(np.asarray(v, np.float32))
    Wk = np.asarray(Wk, np.float32)
    Wq = np.asarray(Wq, np.float32)
    Wv = np.asarray(Wv, np.float32)
    Wo = np.asarray(Wo, np.float32)
    bo = np.asarray(bo, np.float32)
    mask = np.asarray(mask, bool)

    causal_ref = np.triu(np.ones((T, T), bool), k=1)
    if (mask == causal_ref).all():
        causal = True
    elif not mask.any():
        causal = False
    else:
        raise ValueError("only causal or all-false masks are supported")

    nc = _get_nc(causal)
    sel = _sel_matrix()
    dm = _diag_masks() if causal else None

    xq = [np.ascontiguousarray(q[b].T.astype(np.float16)) for b in range(B)]
    xk = [np.ascontiguousarray(k[b].T.astype(np.float16)) for b in range(B)]
    xv = [np.ascontiguousarray(v[b].T.astype(np.float16)) for b in range(B)]

    in_maps = []
    for c in range(8):
        b, g = divmod(c, 4)
        ds = DL * g
        im = {
            "xq": xq[b],
            "xk": xk[b],
            "xv": xv[b],
            "wq": np.ascontiguousarray(Wq[ds:ds + DL, :].T.astype(np.float16)),
            "wk": np.ascontiguousarray(Wk[ds:ds + DL, :].T.astype(np.float16)),
            "wv": np.ascontiguousarray(Wv[ds:ds + DL, :].T.astype(np.float16)),
            "wo": np.ascontiguousarray(Wo[:, ds:ds + DL].T),
            "sel": sel,
            "onescol": np.ones((P, NKT), np.float32),
        }
        if causal:
            im["dmask"] = dm
        in_maps.append(im)

    res = run_bass_kernel_spmd(nc, in_maps, list(range(8)), trace=TRACE)
    LAST_RESULT = res

    out = np.zeros((B, T, E), np.float32)
    for c in range(8):
        b = c // 4
        out[b] += res.results[c]["yt"].T
    out += bo
    return out
